# revision 19
# baseline (speedup 1.0000x reference)
# Causal multi-head attention (B=4, L=2048, H=16, E=64, fp32) on 8 TRN2
# NeuronCores. Sharding: the 64 (b,h) pairs split 8 per core; each core
# computes its pairs fully independently (data parallel on B, tensor
# parallel on H).
#
# v2 design (host-layout + balanced exp pipeline):
#   Host pre-transposes Q,K to [e,l] bf16 and pre-augments V with a ones
#   column, so the device does zero input transposes and zero casts.
#   Per core, heads are processed two at a time (packed into the two
#   64-row halves of the PE array for the score matmuls):
#     S^T[s,l] = K^T . Q   chunks in PSUM (causal-skipped, bf16, dual-issued)
#     P^T = exp(S^T/8)     whole chunks alternate between ScalarE (exact
#                          exp) and VectorE (Schraudolph fast-exp)
#     diagonal tiles masked by an upper-triangular 0/1 multiply on VectorE
#     O^T[d,l] accumulates in PSUM with V (ones-augmented) stationary and
#     P^T streaming; AV matmuls trail the score matmuls by 3 chunks
#   O^T (including the rowsum row from the ones column) is copied to SBUF
#   as bf16 (alternating ScalarE/VectorE) and stored unnormalized; the
#   host divides by the rowsum and transposes back.  This matches the
#   baseline's precision (output was already rounded through bf16).
# L is processed in 4 phases of 512 columns; PSUM = 3 score bufs (6
# banks) + 1 O^T accumulator (2 banks).

import sys

import numpy as np

try:
    import concourse.bass as bass  # noqa: F401
except ImportError:
    sys.path.insert(0, "/opt/trn_rl_repo")

import ml_dtypes

B, L, H, E = 4, 2048, 16, 64
NCORES = 8
BH = B * H                  # 64 (b,h) pairs
BH_PER_CORE = BH // NCORES  # 8
NPAIRS = BH_PER_CORE // 2   # 4 packed pairs per core
NLT = L // 128              # 16 l-tiles
NPH = 4                     # phases over l
PHL = L // NPH              # 512 l-cols per phase
VW = 66                     # V columns + ones col + zero pad

# Schraudolph fast-exp constants for bf16 output:
#   bits_i16 = round((S * scale) * log2(e) * 128 + (127*128 - 128*c))
# with c = 0.0436775 balancing the max relative error to ~±3%.
EXP_A = (1.0 / 8.0) * 1.4426950408889634 * 128.0   # 23.08312...
EXP_B = 127.0 * 128.0 - 5.5907                      # 16250.41

_CACHE = {}


def _phase_chunks(ph):
    """(st, lstart, w) for every s-tile contributing to phase ph."""
    lo, hi = ph * PHL, (ph + 1) * PHL
    return [(st, max(st * 128, lo), hi - max(st * 128, lo)) for st in range(4 * ph + 4)]


def _build_program():
    from contextlib import ExitStack

    import concourse.bass as bass
    import concourse.mybir as mybir
    import concourse.tile as tile
    from concourse import bacc
    from concourse.masks import make_upper_triangular

    f32 = mybir.dt.float32
    bf16 = mybir.dt.bfloat16
    i16 = mybir.dt.int16

    nc = bacc.Bacc(
        "TRN2",
        target_bir_lowering=False,
        debug=False,
        enable_asserts=False,
        num_devices=NCORES,
    )
    # host-prepared layouts:
    #   q/k: [128 rows=(j,e), pair, l]  (already transposed + bf16)
    #   v:   [128 rows=s-in-tile, pair, j, s-tile, VW]  (ones baked in col 64)
    #   o:   [VW rows=(d + rowsum), pair, j, phase, l-in-phase]  (unnormalized)
    q_d = nc.dram_tensor("q", [128, NPAIRS, L], bf16, kind="ExternalInput").ap()
    k_d = nc.dram_tensor("k", [128, NPAIRS, L], bf16, kind="ExternalInput").ap()
    v_d = nc.dram_tensor("v", [128, NPAIRS, 2, NLT, VW], bf16, kind="ExternalInput").ap()
    o_d = nc.dram_tensor("o", [VW, NPAIRS, 2, NPH, PHL], bf16, kind="ExternalOutput").ap()

    with tile.TileContext(nc) as tc, ExitStack() as ctx:
        consts = ctx.enter_context(tc.tile_pool(name="consts", bufs=1))
        qkp = ctx.enter_context(tc.tile_pool(name="qkp", bufs=4))
        vp = ctx.enter_context(tc.tile_pool(name="vp", bufs=4))
        ptp = ctx.enter_context(tc.tile_pool(name="ptp", bufs=8))
        otsbp = ctx.enter_context(tc.tile_pool(name="otsbp", bufs=2))
        spsum = ctx.enter_context(tc.tile_pool(name="spsum", bufs=3, space="PSUM"))
        otps = ctx.enter_context(tc.tile_pool(name="otps", bufs=1, space="PSUM"))

        # mask01[s, j] = 1.0 where s <= j else 0.0 (valid causal region of a
        # diagonal tile of P^T)
        mask01 = consts.tile([128, 128], bf16)
        make_upper_triangular(nc, mask01, val=1.0, diag=True)
        mask01_ap = mask01[:]
        mask01_b = bass.AP(
            tensor=mask01_ap.tensor,
            offset=mask01_ap.offset,
            ap=[mask01_ap.ap[0], [0, 2], mask01_ap.ap[1]],
        )

        scale = 1.0 / float(np.sqrt(E))

        qts, kts, vts = {}, {}, {}

        def load(p, split_first=False):
            qt = qkp.tile([128, L], bf16, tag="qt", name=f"qt{p}")
            kt = qkp.tile([128, L], bf16, tag="kt", name=f"kt{p}")
            vt = vp.tile([128, 2, NLT, VW], bf16, tag="vt", name=f"vt{p}")
            if split_first:
                # first pair: quarters so phase 0 can start as early as
                # possible (phase 0 only needs q/k cols 0:512)
                nc.sync.dma_start(out=qt[:, 0:512], in_=q_d[:, p, 0:512])
                nc.scalar.dma_start(out=kt[:, 0:512], in_=k_d[:, p, 0:512])
                nc.sync.dma_start(out=vt, in_=v_d[:, p])
                nc.scalar.dma_start(out=kt[:, 512:1024], in_=k_d[:, p, 512:1024])
                nc.sync.dma_start(out=qt[:, 512:1024], in_=q_d[:, p, 512:1024])
                nc.scalar.dma_start(out=qt[:, 1024:2048], in_=q_d[:, p, 1024:2048])
                nc.sync.dma_start(out=kt[:, 1024:2048], in_=k_d[:, p, 1024:2048])
            else:
                nc.sync.dma_start(out=qt, in_=q_d[:, p])
                nc.scalar.dma_start(out=kt, in_=k_d[:, p])
                nc.sync.dma_start(out=vt, in_=v_d[:, p])
            qts[p], kts[p], vts[p] = qt, kt, vt

        load(0, split_first=True)

        # warm the PE HAM clock while the first loads are in flight, and
        # trigger the ACT exp table load before the first real activation.
        # A memset-initialized const is ready before mask01's build, so the
        # warmup starts as early as possible.
        warmc = consts.tile([128, 128], bf16)
        nc.gpsimd.memset(warmc, 1.0)
        warm = spsum.tile([128, 1024], f32, tag="sp", name="warm")
        for _ in range(30):
            nc.tensor.matmul(
                out=warm[:, 0:128], lhsT=warmc, rhs=warmc, start=True, stop=True
            )
        warmsb = consts.tile([128, 8], f32)
        nc.vector.tensor_copy(warmsb, warm[:, 0:8])
        warmact = consts.tile([128, 8], bf16)
        nc.scalar.activation(
            warmact, warmsb, mybir.ActivationFunctionType.Exp, scale=0.0
        )

        # greedy elementwise load balance (ns accumulated per engine)
        ew = [0.0, 0.0]  # [scalar, vector]
        pending = []     # deferred epilogue closures from the previous phase

        def emit_pending():
            while pending:
                pending.pop(0)()

        def phase(p, ph, otsb, defer_copy=True):
            qt, kt, vt = qts[p], kts[p], vts[p]
            lo = ph * PHL
            chunks = _phase_chunks(ph)
            nst = len(chunks)
            ots = otps.tile([VW, 2, PHL], f32, tag="ot", name="ot")
            pts = {}

            def emit_av(i):
                st, lstart, w = chunks[i]
                ptt = pts[st]
                for j in range(2):
                    nc.tensor.matmul(
                        out=ots[:, j, lstart - lo : lstart - lo + w],
                        lhsT=vt[:, j, st, :],
                        rhs=ptt[:, j, 0:w],
                        start=(i == 0),
                        stop=(i == nst - 1),
                    )
                del pts[st]

            # AVs trail scores by >=3 chunks and are emitted in groups of G
            # so the PE pays the score<->AV weight-buffer transition once
            # per group instead of twice per chunk.
            G = 3
            next_av = [0]
            masks_pending = []

            def flush_masks():
                while masks_pending:
                    masks_pending.pop(0)()

            def flush_avs(upto):
                flush_masks()
                while next_av[0] <= upto:
                    emit_av(next_av[0])
                    next_av[0] += 1

            for idx, (st, lstart, w) in enumerate(chunks):
                s0 = st * 128
                sp = spsum.tile([128, 1024], f32, tag="sp", name="sp")
                sp2 = sp.rearrange("pp (j c) -> pp j c", j=2)
                for j in range(2):
                    nc.tensor.matmul(
                        out=sp[:, 512 * j : 512 * j + w],
                        lhsT=kt[64 * j : 64 * (j + 1), s0 : s0 + 128],
                        rhs=qt[64 * j : 64 * (j + 1), lstart : lstart + w],
                        start=True,
                        stop=True,
                    )
                pt = ptp.tile([128, 2, PHL], bf16, tag="pt", name="pt")
                diag = lstart == s0
                c_sc = (2 * w + 352) / 1.2
                c_ve = (120 + 2 * w) / 0.96
                if ew[0] + c_sc <= ew[1] + c_ve:
                    ew[0] += c_sc
                    nc.scalar.activation(
                        pt[:, :, 0:w], sp2[:, :, 0:w],
                        mybir.ActivationFunctionType.Exp, scale=scale,
                    )
                else:
                    ew[1] += c_ve
                    nc.vector.tensor_scalar(
                        pt[:, :, 0:w].bitcast(i16),
                        sp2[:, :, 0:w],
                        EXP_A,
                        EXP_B,
                        mybir.AluOpType.mult,
                        mybir.AluOpType.add,
                    )
                flush_masks()
                if diag:
                    # diagonal tile: zero the s > l half (both heads at
                    # once).  Emission is deferred one chunk so the mask
                    # (which may wait on a ScalarE exp) does not sit at the
                    # head of VectorE's queue blocking the next exp.
                    dv = pt[:, :, 0:128]

                    def do_mask(dv=dv):
                        nc.vector.tensor_mul(dv, dv, mask01_b)

                    masks_pending.append(do_mask)
                    ew[1] += 210.0
                pts[st] = pt
                if idx == 2:
                    emit_pending()
                if idx >= 3 and (idx - 3 + 1 - next_av[0]) >= G:
                    flush_avs(idx - 3)
            emit_pending()
            flush_avs(nst - 1)

            # O^T (+ rowsum row) to SBUF as bf16, one head per engine;
            # normalization happens on the host.  Emitted right after the
            # trailing AV batch: the exp engines drain while the PE runs
            # those AVs, so the copies slot into that idle window.
            def copy_out():
                nc.scalar.copy(otsb[:, 0, ph, :], ots[:, 0])
                nc.vector.tensor_copy(otsb[:, 1, ph, :], ots[:, 1])
                ew[0] += 600.0
                ew[1] += 660.0

            copy_out()

        for p in range(NPAIRS):
            if p + 1 < NPAIRS:
                load(p + 1)
            otsb = otsbp.tile([VW, 2, NPH, PHL], bf16, tag="otsb", name=f"otsb{p}")
            if p + 1 < NPAIRS:
                for ph in range(NPH):
                    phase(p, ph, otsb)
                pending.append(
                    lambda p=p, otsb=otsb: nc.sync.dma_start(
                        out=o_d[:, p], in_=otsb
                    )
                )
            else:
                # last pair: biggest phase first so the post-exp tail is
                # short; store each phase as soon as its copy is emitted
                for ph in (3, 2, 1, 0):
                    phase(p, ph, otsb, defer_copy=(ph != 0))
                    pending.append(
                        lambda p=p, ph=ph, otsb=otsb: nc.sync.dma_start(
                            out=o_d[:, p, :, ph, :], in_=otsb[:, :, ph, :]
                        )
                    )
                emit_pending()

    nc.compile()
    return nc


def _get_program():
    if "nc" not in _CACHE:
        _CACHE["nc"] = _build_program()
    return _CACHE["nc"]


def prepare_inputs(q, k, v):
    """Full fp32 [B,L,H,E] tensors -> per-core input maps (host-side
    transpose/pack/cast)."""
    bf = ml_dtypes.bfloat16
    # [B, L, H, E] -> [BH, L, E]
    q_sh = np.ascontiguousarray(q.transpose(0, 2, 1, 3).reshape(BH, L, E))
    k_sh = np.ascontiguousarray(k.transpose(0, 2, 1, 3).reshape(BH, L, E))
    v_sh = np.ascontiguousarray(v.transpose(0, 2, 1, 3).reshape(BH, L, E))
    in_maps = []
    for c in range(NCORES):
        qc = q_sh[c * BH_PER_CORE : (c + 1) * BH_PER_CORE]  # [8, L, E]
        kc = k_sh[c * BH_PER_CORE : (c + 1) * BH_PER_CORE]
        vc = v_sh[c * BH_PER_CORE : (c + 1) * BH_PER_CORE]
        # q/k: [8, L, E] -> [(j e)=128, pair, L]
        qT = np.ascontiguousarray(
            qc.reshape(NPAIRS, 2, L, E).transpose(1, 3, 0, 2).reshape(128, NPAIRS, L)
        ).astype(bf)
        kT = np.ascontiguousarray(
            kc.reshape(NPAIRS, 2, L, E).transpose(1, 3, 0, 2).reshape(128, NPAIRS, L)
        ).astype(bf)
        # v: [8, L, E] -> [128 s-in-tile, pair, j, s-tile, VW]
        vr = vc.reshape(NPAIRS, 2, NLT, 128, E).transpose(3, 0, 1, 2, 4)
        vaug = np.zeros((128, NPAIRS, 2, NLT, VW), dtype=np.float32)
        vaug[..., 0:E] = vr
        vaug[..., E] = 1.0
        in_maps.append({"q": qT, "k": kT, "v": vaug.astype(bf)})
    return in_maps


def postprocess(results):
    """Per-core unnormalized O^T (+rowsum) -> full fp32 [B,L,H,E]."""
    outs = []
    for c in range(NCORES):
        o = np.asarray(results[c]["o"], dtype=np.float32)  # [VW, p, j, ph, PHL]
        x = o.transpose(1, 2, 3, 4, 0).reshape(BH_PER_CORE, L, VW)
        outs.append(x[..., 0:E] / x[..., E : E + 1])
    o_bh = np.concatenate(outs, axis=0)  # [BH, L, E]
    return np.ascontiguousarray(
        o_bh.reshape(B, H, L, E).transpose(0, 2, 1, 3)
    ).astype(np.float32)


def kernel(queries=None, keys=None, values=None, **kw):
    if queries is None or keys is None or values is None:
        raise TypeError("kernel expects queries, keys, values")
    from concourse.bass_utils import run_bass_kernel_spmd

    q = np.asarray(queries, dtype=np.float32)
    k = np.asarray(keys, dtype=np.float32)
    v = np.asarray(values, dtype=np.float32)
    assert q.shape == (B, L, H, E), q.shape

    in_maps = prepare_inputs(q, k, v)
    nc = _get_program()
    res = run_bass_kernel_spmd(nc, in_maps, core_ids=list(range(NCORES)))
    return postprocess(res.results)


if __name__ == "__main__":
    rng = np.random.default_rng(0)
    qq = rng.standard_normal((B, L, H, E), dtype=np.float32)
    kk = rng.standard_normal((B, L, H, E), dtype=np.float32)
    vv = rng.standard_normal((B, L, H, E), dtype=np.float32)
    out = kernel(queries=qq, keys=kk, values=vv)
    print(out.shape, out.dtype)


# revision 20
# speedup vs baseline: 1.0050x; 1.0050x over previous
# Causal multi-head attention (B=4, L=2048, H=16, E=64, fp32) on 8 TRN2
# NeuronCores. Sharding: the 64 (b,h) pairs split 8 per core; each core
# computes its pairs fully independently (data parallel on B, tensor
# parallel on H).
#
# v2 design (host-layout + balanced exp pipeline):
#   Host pre-transposes Q,K to [e,l] bf16 and pre-augments V with a ones
#   column, so the device does zero input transposes and zero casts.
#   Per core, heads are processed two at a time (packed into the two
#   64-row halves of the PE array for the score matmuls):
#     S^T[s,l] = K^T . Q   chunks in PSUM (causal-skipped, bf16, dual-issued)
#     P^T = exp(S^T/8)     whole chunks alternate between ScalarE (exact
#                          exp) and VectorE (Schraudolph fast-exp)
#     diagonal tiles masked by an upper-triangular 0/1 multiply on VectorE
#     O^T[d,l] accumulates in PSUM with V (ones-augmented) stationary and
#     P^T streaming; AV matmuls trail the score matmuls by 3 chunks
#   O^T (including the rowsum row from the ones column) is copied to SBUF
#   as bf16 (alternating ScalarE/VectorE) and stored unnormalized; the
#   host divides by the rowsum and transposes back.  This matches the
#   baseline's precision (output was already rounded through bf16).
# L is processed in 4 phases of 512 columns; PSUM = 3 score bufs (6
# banks) + 1 O^T accumulator (2 banks).

import sys

import numpy as np

try:
    import concourse.bass as bass  # noqa: F401
except ImportError:
    sys.path.insert(0, "/opt/trn_rl_repo")

import ml_dtypes

B, L, H, E = 4, 2048, 16, 64
NCORES = 8
BH = B * H                  # 64 (b,h) pairs
BH_PER_CORE = BH // NCORES  # 8
NPAIRS = BH_PER_CORE // 2   # 4 packed pairs per core
NLT = L // 128              # 16 l-tiles
NPH = 4                     # phases over l
PHL = L // NPH              # 512 l-cols per phase
VW = 66                     # V columns + ones col + zero pad

# Schraudolph fast-exp constants for bf16 output:
#   bits_i16 = round((S * scale) * log2(e) * 128 + (127*128 - 128*c))
# with c = 0.0436775 balancing the max relative error to ~±3%.
EXP_A = (1.0 / 8.0) * 1.4426950408889634 * 128.0   # 23.08312...
EXP_B = 127.0 * 128.0 - 5.5907                      # 16250.41

_CACHE = {}


def _phase_chunks(ph):
    """(st, lstart, w) for every s-tile contributing to phase ph."""
    lo, hi = ph * PHL, (ph + 1) * PHL
    return [(st, max(st * 128, lo), hi - max(st * 128, lo)) for st in range(4 * ph + 4)]


def _build_program():
    from contextlib import ExitStack

    import concourse.bass as bass
    import concourse.mybir as mybir
    import concourse.tile as tile
    from concourse import bacc
    from concourse.masks import make_upper_triangular

    f32 = mybir.dt.float32
    bf16 = mybir.dt.bfloat16
    i16 = mybir.dt.int16

    nc = bacc.Bacc(
        "TRN2",
        target_bir_lowering=False,
        debug=False,
        enable_asserts=False,
        num_devices=NCORES,
    )
    # host-prepared layouts:
    #   q/k: [128 rows=(j,e), pair, l]  (already transposed + bf16)
    #   v:   [128 rows=s-in-tile, pair, j, s-tile, VW]  (ones baked in col 64)
    #   o:   [VW rows=(d + rowsum), pair, j, phase, l-in-phase]  (unnormalized)
    q_d = nc.dram_tensor("q", [128, NPAIRS, L], bf16, kind="ExternalInput").ap()
    k_d = nc.dram_tensor("k", [128, NPAIRS, L], bf16, kind="ExternalInput").ap()
    v_d = nc.dram_tensor("v", [128, NPAIRS, 2, NLT, VW], bf16, kind="ExternalInput").ap()
    o_d = nc.dram_tensor("o", [VW, NPAIRS, 2, NPH, PHL], bf16, kind="ExternalOutput").ap()

    with tile.TileContext(nc) as tc, ExitStack() as ctx:
        consts = ctx.enter_context(tc.tile_pool(name="consts", bufs=1))
        qkp = ctx.enter_context(tc.tile_pool(name="qkp", bufs=4))
        vp = ctx.enter_context(tc.tile_pool(name="vp", bufs=4))
        ptp = ctx.enter_context(tc.tile_pool(name="ptp", bufs=8))
        otsbp = ctx.enter_context(tc.tile_pool(name="otsbp", bufs=2))
        spsum = ctx.enter_context(tc.tile_pool(name="spsum", bufs=3, space="PSUM"))
        otps = ctx.enter_context(tc.tile_pool(name="otps", bufs=1, space="PSUM"))

        # mask01[s, j] = 1.0 where s <= j else 0.0 (valid causal region of a
        # diagonal tile of P^T)
        mask01 = consts.tile([128, 128], bf16)
        make_upper_triangular(nc, mask01, val=1.0, diag=True)
        mask01_ap = mask01[:]
        mask01_b = bass.AP(
            tensor=mask01_ap.tensor,
            offset=mask01_ap.offset,
            ap=[mask01_ap.ap[0], [0, 2], mask01_ap.ap[1]],
        )

        scale = 1.0 / float(np.sqrt(E))

        qts, kts, vts = {}, {}, {}

        def load(p, split_first=False):
            qt = qkp.tile([128, L], bf16, tag="qt", name=f"qt{p}")
            kt = qkp.tile([128, L], bf16, tag="kt", name=f"kt{p}")
            vt = vp.tile([128, 2, NLT, VW], bf16, tag="vt", name=f"vt{p}")
            if split_first:
                # first pair: quarters so phase 0 can start as early as
                # possible (phase 0 only needs q/k cols 0:512)
                nc.sync.dma_start(out=qt[:, 0:512], in_=q_d[:, p, 0:512])
                nc.scalar.dma_start(out=kt[:, 0:512], in_=k_d[:, p, 0:512])
                nc.sync.dma_start(out=vt, in_=v_d[:, p])
                nc.scalar.dma_start(out=kt[:, 512:1024], in_=k_d[:, p, 512:1024])
                nc.sync.dma_start(out=qt[:, 512:1024], in_=q_d[:, p, 512:1024])
                nc.scalar.dma_start(out=qt[:, 1024:2048], in_=q_d[:, p, 1024:2048])
                nc.sync.dma_start(out=kt[:, 1024:2048], in_=k_d[:, p, 1024:2048])
            else:
                nc.sync.dma_start(out=qt, in_=q_d[:, p])
                nc.scalar.dma_start(out=kt, in_=k_d[:, p])
                nc.sync.dma_start(out=vt, in_=v_d[:, p])
            qts[p], kts[p], vts[p] = qt, kt, vt

        load(0, split_first=True)

        # warm the PE HAM clock while the first loads are in flight, and
        # trigger the ACT exp table load before the first real activation.
        # A memset-initialized const is ready before mask01's build, so the
        # warmup starts as early as possible.
        warmc = consts.tile([128, 128], bf16)
        nc.gpsimd.memset(warmc, 1.0)
        warm = spsum.tile([128, 1024], f32, tag="sp", name="warm")
        for _ in range(30):
            nc.tensor.matmul(
                out=warm[:, 0:128], lhsT=warmc, rhs=warmc, start=True, stop=True
            )
        warmsb = consts.tile([128, 8], f32)
        nc.vector.tensor_copy(warmsb, warm[:, 0:8])
        warmact = consts.tile([128, 8], bf16)
        nc.scalar.activation(
            warmact, warmsb, mybir.ActivationFunctionType.Exp, scale=0.0
        )

        # greedy elementwise load balance (ns accumulated per engine)
        ew = [0.0, 0.0]  # [scalar, vector]
        pending = []     # deferred epilogue closures from the previous phase

        def emit_pending():
            while pending:
                pending.pop(0)()

        def phase(p, ph, otsb, defer_copy=True):
            qt, kt, vt = qts[p], kts[p], vts[p]
            lo = ph * PHL
            chunks = _phase_chunks(ph)
            nst = len(chunks)
            ots = otps.tile([VW, 2, PHL], f32, tag="ot", name="ot")
            pts = {}

            def emit_av(i):
                st, lstart, w = chunks[i]
                ptt = pts[st]
                for j in range(2):
                    nc.tensor.matmul(
                        out=ots[:, j, lstart - lo : lstart - lo + w],
                        lhsT=vt[:, j, st, :],
                        rhs=ptt[:, j, 0:w],
                        start=(i == 0),
                        stop=(i == nst - 1),
                    )
                del pts[st]

            # AVs trail scores by >=3 chunks and are emitted in groups of G
            # so the PE pays the score<->AV weight-buffer transition once
            # per group instead of twice per chunk.
            G = 3
            next_av = [0]
            masks_pending = []

            def flush_masks():
                while masks_pending:
                    masks_pending.pop(0)()

            def flush_avs(upto):
                flush_masks()
                while next_av[0] <= upto:
                    emit_av(next_av[0])
                    next_av[0] += 1

            for idx, (st, lstart, w) in enumerate(chunks):
                s0 = st * 128
                sp = spsum.tile([128, 1024], f32, tag="sp", name="sp")
                sp2 = sp.rearrange("pp (j c) -> pp j c", j=2)
                for j in range(2):
                    nc.tensor.matmul(
                        out=sp[:, 512 * j : 512 * j + w],
                        lhsT=kt[64 * j : 64 * (j + 1), s0 : s0 + 128],
                        rhs=qt[64 * j : 64 * (j + 1), lstart : lstart + w],
                        start=True,
                        stop=True,
                    )
                pt = ptp.tile([128, 2, PHL], bf16, tag="pt", name="pt")
                diag = lstart == s0
                c_sc = (2 * w + 352) / 1.2
                c_ve = (120 + 2 * w) / 0.96
                if ew[0] + c_sc <= ew[1] + c_ve:
                    ew[0] += c_sc
                    nc.scalar.activation(
                        pt[:, :, 0:w], sp2[:, :, 0:w],
                        mybir.ActivationFunctionType.Exp, scale=scale,
                    )
                else:
                    ew[1] += c_ve
                    nc.vector.tensor_scalar(
                        pt[:, :, 0:w].bitcast(i16),
                        sp2[:, :, 0:w],
                        EXP_A,
                        EXP_B,
                        mybir.AluOpType.mult,
                        mybir.AluOpType.add,
                    )
                flush_masks()
                if diag:
                    # diagonal tile: zero the s > l half (both heads at
                    # once).  Emission is deferred one chunk so the mask
                    # (which may wait on a ScalarE exp) does not sit at the
                    # head of VectorE's queue blocking the next exp.
                    dv = pt[:, :, 0:128]

                    def do_mask(dv=dv):
                        nc.vector.tensor_mul(dv, dv, mask01_b)

                    masks_pending.append(do_mask)
                    ew[1] += 210.0
                pts[st] = pt
                if idx == 2:
                    emit_pending()
                if idx >= 3 and (idx - 3 + 1 - next_av[0]) >= G:
                    flush_avs(idx - 3)
            emit_pending()
            flush_avs(nst - 1)

            # O^T (+ rowsum row) to SBUF as bf16, one head per engine;
            # normalization happens on the host.  Deferred into the next
            # phase so it does not stall the exp ping-pong.
            def copy_out():
                nc.scalar.copy(otsb[:, 0, ph, :], ots[:, 0])
                nc.vector.tensor_copy(otsb[:, 1, ph, :], ots[:, 1])
                ew[0] += 600.0
                ew[1] += 660.0

            if defer_copy:
                pending.append(copy_out)
            else:
                copy_out()

        for p in range(NPAIRS):
            if p + 1 < NPAIRS:
                load(p + 1)
            otsb = otsbp.tile([VW, 2, NPH, PHL], bf16, tag="otsb", name=f"otsb{p}")
            if p + 1 < NPAIRS:
                for ph in range(NPH):
                    phase(p, ph, otsb)
                pending.append(
                    lambda p=p, otsb=otsb: nc.sync.dma_start(
                        out=o_d[:, p], in_=otsb
                    )
                )
            else:
                # last pair: biggest phase first so the post-exp tail is
                # short; store each phase as soon as its copy is emitted
                for ph in (3, 2, 1, 0):
                    phase(p, ph, otsb, defer_copy=(ph != 0))
                    pending.append(
                        lambda p=p, ph=ph, otsb=otsb: nc.sync.dma_start(
                            out=o_d[:, p, :, ph, :], in_=otsb[:, :, ph, :]
                        )
                    )
                emit_pending()

    nc.compile()
    return nc


def _get_program():
    if "nc" not in _CACHE:
        _CACHE["nc"] = _build_program()
    return _CACHE["nc"]


def prepare_inputs(q, k, v):
    """Full fp32 [B,L,H,E] tensors -> per-core input maps (host-side
    transpose/pack/cast)."""
    bf = ml_dtypes.bfloat16
    # [B, L, H, E] -> [BH, L, E]
    q_sh = np.ascontiguousarray(q.transpose(0, 2, 1, 3).reshape(BH, L, E))
    k_sh = np.ascontiguousarray(k.transpose(0, 2, 1, 3).reshape(BH, L, E))
    v_sh = np.ascontiguousarray(v.transpose(0, 2, 1, 3).reshape(BH, L, E))
    in_maps = []
    for c in range(NCORES):
        qc = q_sh[c * BH_PER_CORE : (c + 1) * BH_PER_CORE]  # [8, L, E]
        kc = k_sh[c * BH_PER_CORE : (c + 1) * BH_PER_CORE]
        vc = v_sh[c * BH_PER_CORE : (c + 1) * BH_PER_CORE]
        # q/k: [8, L, E] -> [(j e)=128, pair, L]
        qT = np.ascontiguousarray(
            qc.reshape(NPAIRS, 2, L, E).transpose(1, 3, 0, 2).reshape(128, NPAIRS, L)
        ).astype(bf)
        kT = np.ascontiguousarray(
            kc.reshape(NPAIRS, 2, L, E).transpose(1, 3, 0, 2).reshape(128, NPAIRS, L)
        ).astype(bf)
        # v: [8, L, E] -> [128 s-in-tile, pair, j, s-tile, VW]
        vr = vc.reshape(NPAIRS, 2, NLT, 128, E).transpose(3, 0, 1, 2, 4)
        vaug = np.zeros((128, NPAIRS, 2, NLT, VW), dtype=np.float32)
        vaug[..., 0:E] = vr
        vaug[..., E] = 1.0
        in_maps.append({"q": qT, "k": kT, "v": vaug.astype(bf)})
    return in_maps


def postprocess(results):
    """Per-core unnormalized O^T (+rowsum) -> full fp32 [B,L,H,E]."""
    outs = []
    for c in range(NCORES):
        o = np.asarray(results[c]["o"], dtype=np.float32)  # [VW, p, j, ph, PHL]
        x = o.transpose(1, 2, 3, 4, 0).reshape(BH_PER_CORE, L, VW)
        outs.append(x[..., 0:E] / x[..., E : E + 1])
    o_bh = np.concatenate(outs, axis=0)  # [BH, L, E]
    return np.ascontiguousarray(
        o_bh.reshape(B, H, L, E).transpose(0, 2, 1, 3)
    ).astype(np.float32)


def kernel(queries=None, keys=None, values=None, **kw):
    if queries is None or keys is None or values is None:
        raise TypeError("kernel expects queries, keys, values")
    from concourse.bass_utils import run_bass_kernel_spmd

    q = np.asarray(queries, dtype=np.float32)
    k = np.asarray(keys, dtype=np.float32)
    v = np.asarray(values, dtype=np.float32)
    assert q.shape == (B, L, H, E), q.shape

    in_maps = prepare_inputs(q, k, v)
    nc = _get_program()
    res = run_bass_kernel_spmd(nc, in_maps, core_ids=list(range(NCORES)))
    return postprocess(res.results)


if __name__ == "__main__":
    rng = np.random.default_rng(0)
    qq = rng.standard_normal((B, L, H, E), dtype=np.float32)
    kk = rng.standard_normal((B, L, H, E), dtype=np.float32)
    vv = rng.standard_normal((B, L, H, E), dtype=np.float32)
    out = kernel(queries=qq, keys=kk, values=vv)
    print(out.shape, out.dtype)
